# revision 11
# baseline (speedup 1.0000x reference)
"""PeerHTC Trainium2 Bass kernel.

Strategy: data-parallel over batch on 8 NeuronCores (8 batches each).
Label-tree LSTM (tiny) replicated on every core; out_W replicated in fp16.
Host side only re-lays-out inputs (transpose / pad / dtype cast / shard).

Device layouts (per core, B_local=8, 4096 tokens):
  xT       [768, 4096]  text_hidden slice, pre-transposed on host
  textT    [128, 2, 4096] sbuf: tanh(tt_W.T @ xT + b), h on partitions
  textaug  [128, 32, 257] sbuf: text in token-major chunks + ones column
  attT     exp(text_chunk @ leT + pad_bias) per (batch, s_chunk)  [128s, 141]
  pooling  featT = ET.T @ textaug -> [l, 257]; col 256 = softmax denom
  head     logits[8,141] = sum_c FT[:,c,:].T @ Wout[:,c,:]  (282 psum-accum)
"""

import numpy as np

import concourse.bass as bass
import concourse.tile as tile
from concourse import bacc, mybir
from concourse.bass_utils import run_bass_kernel_spmd
from concourse.masks import make_identity

F32 = mybir.dt.float32
F32R = mybir.dt.float32r
F16 = mybir.dt.float16
AF = mybir.ActivationFunctionType
ALU = mybir.AluOpType

H, L, N1, N2, N3 = 256, 141, 9, 34, 98
B, S = 64, 512
NCORES = 8
BL = B // NCORES          # 8 batches per core
TOK = BL * S              # 4096 tokens per core
NT = TOK // 512           # 8 token tiles of 512
NWC = 282                 # out_W 128-row chunks (36096/128)
NEG = -1.0e30

LSTM_W = ["wi1", "wf1", "wo1", "wu1", "wi2", "wf2", "wo2", "wu2"]
LSTM_U = ["ui1", "uf1", "uo1", "uu1", "ui2", "uf2", "uo2", "uu2"]

# fp32r for the big matmuls (moving dim >= 256 -> full PE rate)
R_BIG = True
# fp16 for out_W head
HEAD_F16 = True


RB = F32R if R_BIG else F32


def _r(ap):
    return ap


def pad128(a, rows=128):
    a = np.asarray(a, np.float32)
    out = np.zeros((rows,) + a.shape[1:], np.float32)
    out[: a.shape[0]] = a
    return out


def kxm(W, kc):
    """[kc*128, M] -> [128, kc, M] (contraction chunks on partitions)."""
    W = np.asarray(W, np.float32)
    return np.ascontiguousarray(W.reshape(kc, 128, W.shape[1]).transpose(1, 0, 2))


def bias2(b):
    return np.ascontiguousarray(np.asarray(b, np.float32).reshape(2, 128, 1).transpose(1, 0, 2))


def build_nc():
    nc = bacc.Bacc("TRN2", target_bir_lowering=False, debug=False,
                   num_devices=NCORES)

    d = {}

    def din(name, shape, dt=F32):
        d[name] = nc.dram_tensor(name, list(shape), dt, kind="ExternalInput").ap()
        return d[name]

    din("xT", [768, TOK], RB)
    din("toks", [128, BL, 4])                 # token ids as f32, [s%128, b, s//128]
    din("leT", [128, 2, L])                   # label_encoding.T
    din("le_k0", [128, 256])                  # label_encoding rows 0:128
    din("le_k1", [128, 256])                  # rows 128:141 zero-padded
    din("v1n", [128, 256])
    din("v2n", [128, 256])
    din("v3n", [128, 256])
    for n in LSTM_W + LSTM_U:
        din(n, [128, 2, 256])
    for n in LSTM_W:
        din(n + "b", [128, 2, 1])
    din("w12p", [128, N2])      # w12 padded (K=9)
    din("w23p", [128, N3])      # w23 padded (K=34)
    din("w12Tp", [128, N1])     # w12.T padded (K=34)
    din("w23Tp", [128, N2])     # w23.T padded (K=98)
    din("fre12Tp", [128, N1])   # fre12.T padded (K=34)
    din("fre23Tp", [128, N2])   # fre23.T padded (K=98)
    din("ATk0", [128, L])       # A.T rows 0:128
    din("ATk1", [128, L])       # A.T rows 128:141 padded
    din("Wp", [128, 2, 256])
    din("mixW", [128, 6, 256])
    din("mixb", [128, 2, 1])
    din("ttW", [128, 6, 256], RB)
    din("ttb", [128, 2, 1])
    din("ttb_row", [1, 256], RB)
    din("ones_row", [1, 128], RB)
    din("zpad", [128, 2, 256 - L], RB)
    din("ones_col", [128, 4, 2], RB)
    din("Wout", [128, NWC, L], F16 if HEAD_F16 else F32)
    din("outb", [BL, L])
    out_d = nc.dram_tensor("out", [BL, L], F32, kind="ExternalOutput").ap()

    with tile.TileContext(nc) as tc:
        _emit(nc, tc, d, out_d)
    nc.compile()
    return nc


def _emit(nc, tc, d, out_d):
    import contextlib
    ctx = contextlib.ExitStack()
    with ctx:
        prm = ctx.enter_context(tc.tile_pool(name="prm", bufs=1))
        lab = ctx.enter_context(tc.tile_pool(name="lab", bufs=1))
        xp = ctx.enter_context(tc.tile_pool(name="xp", bufs=2))
        big = ctx.enter_context(tc.tile_pool(name="big", bufs=1))
        etp = ctx.enter_context(tc.tile_pool(name="etp", bufs=2))
        ftp = ctx.enter_context(tc.tile_pool(name="ftp", bufs=2))
        wop = ctx.enter_context(tc.tile_pool(name="wop", bufs=3))
        pp = ctx.enter_context(tc.tile_pool(name="pp", bufs=8, space="PSUM"))

        def ps(pl, fr):
            t = pp.tile([128, 512], F32, tag="ps", name="ps")
            return t[:pl, :fr]

        # ---------------- load params ----------------
        P = {}
        for name, ap in d.items():
            if name in ("xT", "Wout"):
                continue
            t = prm.tile(list(ap.shape), ap.dtype, tag="prm_" + name)
            nc.sync.dma_start(t[:], ap[:])
            P[name] = t
        ident = prm.tile([128, 128], F32, tag="ident")
        make_identity(nc, ident[:])
        ones_row = P["ones_row"]

        # pad bias from token ids: -1e30 where id in {0,101,102}
        pb = lab.tile([128, BL, 4], F32, tag="pb")
        m0 = lab.tile([128, BL, 4], F32, tag="m0")
        toks = P["toks"]
        nc.vector.tensor_scalar(pb[:], toks[:], 0.0, None, ALU.is_equal)
        nc.vector.tensor_scalar(m0[:], toks[:], 101.0, None, ALU.is_equal)
        nc.vector.tensor_add(pb[:], pb[:], m0[:])
        nc.vector.tensor_scalar(m0[:], toks[:], 102.0, None, ALU.is_equal)
        nc.vector.tensor_add(pb[:], pb[:], m0[:])
        nc.vector.tensor_scalar(pb[:], pb[:], NEG, None, ALU.mult)

        # ---------------- label side (tree LSTM), all T-form [128,2,n] ------
        leT = lab.tile([128, 2, 256], RB, tag="leTfin")  # final le.T, padded
        nc.sync.dma_start(leT[:, :, L:], d["zpad"][:])
        le_cat = lab.tile([128, 6, L], F32, tag="le_cat")
        vT = P["leT"]

        def vslice(lo, n):
            return vT[:, :, lo:lo + n]

        def gateT(name, w, xT_ap, n, func, extra=None):
            out = lab.tile([128, 2, n], F32, tag="g_" + name)
            for mc in range(2):
                p = ps(128, n)
                for kc in range(2):
                    nc.tensor.matmul(p, lhsT=P[w][:, kc, mc * 128:(mc + 1) * 128],
                                     rhs=xT_ap[:, kc, :n],
                                     start=(kc == 0),
                                     stop=(kc == 1 and extra is None))
                if extra is not None:
                    u, hT = extra
                    for kc in range(2):
                        nc.tensor.matmul(p, lhsT=u[:, kc, mc * 128:(mc + 1) * 128],
                                         rhs=hT[:, kc, :n],
                                         start=False, stop=(kc == 1))
                nc.scalar.activation(out[:, mc, :], p, func,
                                     bias=P[w + "b"][:, mc, :])
            return out

        def to_nat(tT, n, name):
            nat = lab.tile([128, 256], F32, tag="n_" + name)
            nc.vector.memset(nat[:], 0.0)
            for mc in range(2):
                p = ps(n, 128)
                nc.tensor.transpose(p, tT[:, mc, :n], ident[:])
                nc.vector.tensor_copy(out=nat[:n, mc * 128:(mc + 1) * 128], in_=p)
            return nat

        def matT(natA, rhsP, m, name):
            """(M @ X).T given lhsT=natA (X natural, K-padded) and rhsP=M.T padded."""
            out = lab.tile([128, 2, m], F32, tag="t_" + name)
            for mc in range(2):
                p = ps(128, m)
                nc.tensor.matmul(p, lhsT=natA[:, mc * 128:(mc + 1) * 128],
                                 rhs=rhsP[:, :m], start=True, stop=True)
                nc.vector.tensor_copy(out=out[:, mc, :], in_=p)
            return out

        def emul(a, b, n, name, out=None):
            o = out if out is not None else lab.tile([128, 2, n], F32, tag="e_" + name)
            nc.vector.tensor_mul(o[:, :, :n] if out is not None else o[:],
                                 a[:, :, :n], b[:, :, :n])
            return o

        def tanh_t(a, n, name):
            o = lab.tile([128, 2, n], F32, tag="th_" + name)
            nc.scalar.activation(o[:], a[:, :, :n], AF.Tanh)
            return o

        Sig, Tanh = AF.Sigmoid, AF.Tanh

        # top-down level 1
        i11 = gateT("i11", "wi1", vslice(0, N1), N1, Sig)
        o11 = gateT("o11", "wo1", vslice(0, N1), N1, Sig)
        u11 = gateT("u11", "wu1", vslice(0, N1), N1, Tanh)
        c11 = emul(i11, u11, N1, "c11")
        h11 = le_cat[:, 0:2, 0:N1]
        emul(o11, tanh_t(c11, N1, "c11"), N1, "h11", out=h11)
        h11n = to_nat(h11, N1, "h11")
        c11n = to_nat(c11, N1, "c11")
        # level 2 top-down
        h21t = matT(h11n, P["w12p"], N2, "h21t")
        c21p = matT(c11n, P["w12p"], N2, "c21p")
        i21 = gateT("i21", "wi1", vslice(N1, N2), N2, Sig, (P["ui1"], h21t))
        f21 = gateT("f21", "wf1", vslice(N1, N2), N2, Sig, (P["uf1"], h21t))
        o21 = gateT("o21", "wo1", vslice(N1, N2), N2, Sig, (P["uo1"], h21t))
        u21 = gateT("u21", "wu1", vslice(N1, N2), N2, Tanh, (P["uu1"], h21t))
        c21 = emul(i21, u21, N2, "c21")
        nc.vector.tensor_mul(f21[:], f21[:], c21p[:])
        nc.vector.tensor_add(c21[:], c21[:], f21[:])
        h21 = le_cat[:, 0:2, N1:N1 + N2]
        emul(o21, tanh_t(c21, N2, "c21"), N2, "h21", out=h21)
        h21n = to_nat(h21, N2, "h21")
        c21n = to_nat(c21, N2, "c21")
        # level 3 top-down
        h31t = matT(h21n, P["w23p"], N3, "h31t")
        c31p = matT(c21n, P["w23p"], N3, "c31p")
        i31 = gateT("i31", "wi1", vslice(N1 + N2, N3), N3, Sig, (P["ui1"], h31t))
        f31 = gateT("f31", "wf1", vslice(N1 + N2, N3), N3, Sig, (P["uf1"], h31t))
        o31 = gateT("o31", "wo1", vslice(N1 + N2, N3), N3, Sig, (P["uo1"], h31t))
        u31 = gateT("u31", "wu1", vslice(N1 + N2, N3), N3, Tanh, (P["uu1"], h31t))
        c31 = emul(i31, u31, N3, "c31")
        nc.vector.tensor_mul(f31[:], f31[:], c31p[:])
        nc.vector.tensor_add(c31[:], c31[:], f31[:])
        h31 = le_cat[:, 0:2, N1 + N2:L]
        emul(o31, tanh_t(c31, N3, "c31"), N3, "h31", out=h31)
        # bottom-up level 3
        i32 = gateT("i32", "wi2", vslice(N1 + N2, N3), N3, Sig)
        o32 = gateT("o32", "wo2", vslice(N1 + N2, N3), N3, Sig)
        u32 = gateT("u32", "wu2", vslice(N1 + N2, N3), N3, Tanh)
        c32 = emul(i32, u32, N3, "c32")
        h32 = le_cat[:, 2:4, N1 + N2:L]
        emul(o32, tanh_t(c32, N3, "c32"), N3, "h32", out=h32)
        h32n = to_nat(h32, N3, "h32")
        # bottom-up level 2
        h22t = matT(h32n, P["fre23Tp"], N2, "h22t")
        wv2 = matT(P["v2n"], P["w23p"], N3, "wv2")
        i22 = gateT("i22", "wi2", vslice(N1, N2), N2, Sig, (P["ui2"], h22t))
        o22 = gateT("o22", "wo2", vslice(N1, N2), N2, Sig, (P["uo2"], h22t))
        u22 = gateT("u22", "wu2", vslice(N1, N2), N2, Tanh, (P["uu2"], h22t))
        f22 = gateT("f22", "wf2", wv2, N3, Sig, (P["uf2"], h32))
        z22 = emul(f22, c32, N3, "z22")
        z22n = to_nat(z22, N3, "z22")
        w23z = matT(z22n, P["w23Tp"], N2, "w23z")
        c22 = emul(i22, u22, N2, "c22")
        nc.vector.tensor_add(c22[:], c22[:], w23z[:])
        h22 = le_cat[:, 2:4, N1:N1 + N2]
        emul(o22, tanh_t(c22, N2, "c22"), N2, "h22", out=h22)
        h22n = to_nat(h22, N2, "h22")
        # bottom-up level 1
        h12t = matT(h22n, P["fre12Tp"], N1, "h12t")
        wv1 = matT(P["v1n"], P["w12p"], N2, "wv1")
        i12 = gateT("i12", "wi2", vslice(0, N1), N1, Sig, (P["ui2"], h12t))
        o12 = gateT("o12", "wo2", vslice(0, N1), N1, Sig, (P["uo2"], h12t))
        u12 = gateT("u12", "wu2", vslice(0, N1), N1, Tanh, (P["uu2"], h12t))
        f12 = gateT("f12", "wf2", wv1, N2, Sig, (P["uf2"], h22))
        z12 = emul(f12, c22, N2, "z12")
        z12n = to_nat(z12, N2, "z12")
        w12z = matT(z12n, P["w12Tp"], N1, "w12z")
        c12 = emul(i12, u12, N1, "c12")
        nc.vector.tensor_add(c12[:], c12[:], w12z[:])
        h12 = le_cat[:, 2:4, 0:N1]
        emul(o12, tanh_t(c12, N1, "c12"), N1, "h12", out=h12)
        # temp = relu(A @ LE @ Wp), T-form into le_cat[:, 4:6, :]
        st = lab.tile([128, 2, L], F32, tag="st")
        for mc in range(2):
            p = ps(128, L)
            nc.tensor.matmul(p, lhsT=P["le_k0"][:, mc * 128:(mc + 1) * 128],
                             rhs=P["ATk0"][:], start=True, stop=False)
            nc.tensor.matmul(p, lhsT=P["le_k1"][:, mc * 128:(mc + 1) * 128],
                             rhs=P["ATk1"][:], start=False, stop=True)
            nc.vector.tensor_copy(out=st[:, mc, :], in_=p)
        for mc in range(2):
            p = ps(128, L)
            for kc in range(2):
                nc.tensor.matmul(p, lhsT=P["Wp"][:, kc, mc * 128:(mc + 1) * 128],
                                 rhs=st[:, kc, :], start=(kc == 0), stop=(kc == 1))
            nc.scalar.activation(le_cat[:, 4 + mc, :], p, AF.Relu)
        # le = le_cat @ mix_W + mix_b  (T-form)
        for mc in range(2):
            p = ps(128, L)
            for kc in range(6):
                nc.tensor.matmul(p, lhsT=P["mixW"][:, kc, mc * 128:(mc + 1) * 128],
                                 rhs=le_cat[:, kc, :], start=(kc == 0), stop=(kc == 5))
            nc.scalar.activation(leT[:, mc, :L], p, AF.Identity,
                                 bias=P["mixb"][:, mc, :])

        # ---------------- text + attention + pooling, one batch per tile ----
        FT = big.tile([128, L, 2, BL], F16 if HEAD_F16 else F32, tag="FT")
        xT_ap = d["xT"].rearrange("(kc p) (n s) -> n p kc s", p=128, s=512)

        for b in range(BL):
            xt = xp.tile([128, 6, 512], RB, tag="xt")
            nc.sync.dma_start(xt[:], xT_ap[b])
            # textT chunk: tanh(ttW.T @ xT + b)   [128h, 2, 512]
            textT = xp.tile([128, 2, 512], RB, tag="textT")
            for hc in range(2):
                p = ps(128, 512)
                for kc in range(6):
                    nc.tensor.matmul(p, lhsT=_r(P["ttW"][:, kc, hc * 128:(hc + 1) * 128]),
                                     rhs=_r(xt[:, kc, :]),
                                     start=(kc == 0), stop=(kc == 5))
                nc.scalar.activation(textT[:, hc, :], p,
                                     AF.Tanh, bias=P["ttb"][:, hc, :])
            # token-major text (+ bias via rank-1 ones matmul), ones col at 256
            textaug = xp.tile([128, 4, 258], RB, tag="textaug")
            nc.sync.dma_start(textaug[:, :, 256:258], d["ones_col"][:])
            for sc in range(4):
                p = ps(128, 256)
                for kc in range(6):
                    nc.tensor.matmul(p, lhsT=_r(xt[:, kc, sc * 128:(sc + 1) * 128]),
                                     rhs=_r(P["ttW"][:, kc, :]),
                                     start=(kc == 0), stop=False)
                nc.tensor.matmul(p, lhsT=_r(ones_row[:, :128]),
                                 rhs=_r(P["ttb_row"][:]),
                                 start=False, stop=True)
                nc.scalar.activation(textaug[:, sc, :256], p, AF.Tanh)

            et = etp.tile([128, 4, L], RB, tag="et")
            for sc in range(4):
                p = ps(128, 256)
                for hc in range(2):
                    nc.tensor.matmul(p, lhsT=_r(textT[:, hc, sc * 128:(sc + 1) * 128]),
                                     rhs=_r(leT[:, hc, :]),
                                     start=(hc == 0), stop=(hc == 1))
                nc.scalar.activation(et[:, sc, :], p[:, :L], AF.Exp,
                                     bias=pb[:, b, sc:sc + 1])
            feat = ftp.tile([128, 2, 256], F32, tag="feat")
            rec = ftp.tile([128, 2, 1], F32, tag="rec")
            for lc in range(2):
                pl = 128 if lc == 0 else L - 128
                p = ps(pl, 258)
                for sc in range(4):
                    nc.tensor.matmul(p, lhsT=_r(et[:, sc, lc * 128:lc * 128 + pl]),
                                     rhs=_r(textaug[:, sc, :]),
                                     start=(sc == 0), stop=(sc == 3))
                nc.vector.reciprocal(rec[:pl, lc, :], p[:, 256:257])
                nc.vector.tensor_scalar(feat[:pl, lc, :], p[:, :256],
                                        rec[:pl, lc, :], None, ALU.mult)
            for lc in range(2):
                pl = 128 if lc == 0 else L - 128
                for hc in range(2):
                    p = ps(128, 128)
                    nc.tensor.transpose(p, feat[:, lc, hc * 128:(hc + 1) * 128],
                                        ident[:])
                    nc.vector.tensor_copy(out=FT[:, lc * 128:lc * 128 + pl, hc, b],
                                          in_=p[:, :pl])

        # ---------------- output head ----------------
        wdt = F16 if HEAD_F16 else F32
        php = ps(BL, L)
        gs = 0
        wout_ap = d["Wout"]
        while gs < NWC:
            gn = min(24, NWC - gs)
            wt = wop.tile([128, 24, L], wdt, tag="wout")
            nc.sync.dma_start(wt[:, :gn, :], wout_ap[:, gs:gs + gn, :])
            for ci in range(gn):
                c = gs + ci
                nc.tensor.matmul(php, lhsT=FT[:, c // 2, c % 2, :],
                                 rhs=wt[:, ci, :],
                                 start=(c == 0), stop=(c == NWC - 1))
            gs += gn
        res = lab.tile([BL, L], F32, tag="res")
        nc.vector.tensor_add(res[:], php, P["outb"][:])
        nc.scalar.activation(res[:], res[:], AF.Sigmoid)
        nc.sync.dma_start(out_d[:], res[:])


# ---------------------------------------------------------------------------
# host side
# ---------------------------------------------------------------------------

def _shared_params(inputs):
    g = {k: np.asarray(v) for k, v in inputs.items()}
    LE = np.asarray(g["label_encoding"], np.float32)
    sh = {}
    sh["leT"] = np.ascontiguousarray(LE.T.reshape(2, 128, L).transpose(1, 0, 2))
    sh["le_k0"] = np.ascontiguousarray(LE[:128])
    sh["le_k1"] = pad128(LE[128:L])
    sh["v1n"] = pad128(LE[:N1])
    sh["v2n"] = pad128(LE[N1:N1 + N2])
    sh["v3n"] = pad128(LE[N1 + N2:])
    for n in LSTM_W + LSTM_U:
        sh[n] = kxm(g[n + "_W"], 2)
    for n in LSTM_W:
        sh[n + "b"] = bias2(g[n + "_b"])
    w12 = np.asarray(g["w12"], np.float32)
    w23 = np.asarray(g["w23"], np.float32)
    sh["w12p"] = pad128(w12)
    sh["w23p"] = pad128(w23)
    sh["w12Tp"] = pad128(w12.T)
    sh["w23Tp"] = pad128(w23.T)
    sh["fre12Tp"] = pad128(np.asarray(g["fre12"], np.float32).T)
    sh["fre23Tp"] = pad128(np.asarray(g["fre23"], np.float32).T)
    AT = np.asarray(g["A"], np.float32).T
    sh["ATk0"] = np.ascontiguousarray(AT[:128])
    sh["ATk1"] = pad128(AT[128:L])
    sh["Wp"] = kxm(g["Wp"], 2)
    sh["mixW"] = kxm(g["mix_W"], 6)
    sh["mixb"] = bias2(g["mix_b"])
    sh["ttW"] = kxm(g["tt_W"], 6)
    sh["ttb"] = bias2(g["tt_b"])
    sh["ttb_row"] = np.asarray(g["tt_b"], np.float32).reshape(1, 256).copy()
    sh["ones_row"] = np.ones((1, 128), np.float32)
    sh["zpad"] = np.zeros((128, 2, 256 - L), np.float32)
    oc = np.zeros((128, 4, 2), np.float32); oc[:, :, 0] = 1.0
    sh["ones_col"] = oc
    ow = np.asarray(g["out_W"], np.float32).reshape(NWC, 128, L).transpose(1, 0, 2)
    sh["Wout"] = np.ascontiguousarray(ow.astype(np.float16 if HEAD_F16 else np.float32))
    sh["outb"] = np.ascontiguousarray(
        np.tile(np.asarray(g["out_b"], np.float32)[None, :], (BL, 1)))
    return sh


def make_in_maps(inputs):
    sh = _shared_params(inputs)
    th = np.asarray(inputs["text_hidden"], np.float32)
    toks = np.asarray(inputs["inputs"]).astype(np.float32)
    maps = []
    for c in range(NCORES):
        m = dict(sh)
        m["xT"] = np.ascontiguousarray(
            th[c * BL:(c + 1) * BL].reshape(TOK, 768).T)
        m["toks"] = np.ascontiguousarray(
            toks[c * BL:(c + 1) * BL].reshape(BL, 4, 128).transpose(2, 0, 1))
        maps.append(m)
    return maps


_NC_CACHE = {}


def get_nc():
    if "nc" not in _NC_CACHE:
        _NC_CACHE["nc"] = build_nc()
    return _NC_CACHE["nc"]


def kernel(**inputs):
    nc = get_nc()
    maps = make_in_maps(inputs)
    res = run_bass_kernel_spmd(nc, maps, core_ids=list(range(NCORES)))
    out = np.concatenate([r["out"] for r in res.results], axis=0)
    return np.ascontiguousarray(out.astype(np.float32))
